# revision 1
# baseline (speedup 1.0000x reference)
"""Trainium2 Bass kernel for nn_ContrastiveLoss (B=4096, D=256, margin=1.0).

Math (exact restructuring of the reference):
  loss = [ sum_{i<j, same} 0.5*(d2_ij + 1e-8)
         + sum_{i<j, diff} 0.5*relu(1 - d_ij)^2 ] / (B(B-1)/2 + 1e-8)

  The similar-pair term has a closed form per class c:
     sum_{i<j in c} d2 = n_c * sum_sq_c - ||sum_e_c||^2
  so it only needs class sums / summed squared norms (computed on device).

  The dissimilar term needs elementwise distances only on the mixed-label
  (n_small x n_large) rectangle.  Rows are sorted by label on host; each of
  the 8 cores gets a (R_CAP/2 x C_CAP/4) block.  relu(1-d)^2 is EXACTLY zero
  unless some mixed pair has d2 < 1, so the fast program only has to PROVE
  no pair violates the margin: the GEMM leaves psum = dot_ij - 0.5*sq_i and
  a single DVE tensor_scalar per psum block computes
     accum[j] = max( max_i(psum[j,i] - 0.5*sq_j), -1.0 )   (= -0.5*min(d2,2))
  If every accum <= -0.7 (d2_min >= 1.4 with margin for bf16 noise), the
  dissimilar term is exactly 0.  Otherwise a full fallback program (sqrt
  pipeline, compiled lazily) recomputes it exactly.

Pad rows use zero embeddings (class sums unpolluted) and get +BIG added to
their squared norm via the augmentation terms, pushing their distances far
above the margin.
"""

import sys
import os

for _p in ("/opt/trn_rl_repo", "/root/.axon_site/_ro/trn_rl_repo"):
    if os.path.isdir(_p) and _p not in sys.path:
        sys.path.insert(0, _p)

import numpy as np

B_FULL, D = 4096, 256
MARGIN = 1.0
EPS = 1e-8
BIG = 1.0e4
R_CAP, C_CAP = 2048, 2560      # padded small-class rows / large-class cols
RSH, CSH = 2, 4                # core grid: row-shards x col-shards
AR = R_CAP // RSH              # 1024 rectangle rows per core (free axis)
BC = C_CAP // CSH              # 768 rectangle cols per core (partition axis)
NBLK = BC // 128               # 6 psum blocks per core
N_CORES = 8

# detection threshold: trigger the exact fallback if min mixed d2 < 1.4
DETECT_ACCUM_THRESH = -0.7
LAMB = 0.25                    # exp-bound sharpness for ACT-side detection
EXP_BLOCKS = (0, 2)            # blocks detected via ACT exp-sum bound
MAX_BLOCKS = (1, 3, 4)         # blocks detected via DVE max-reduce

_PROGRAMS = {}


def _build_detect_program():
    """Fast path: GEMM + margin-violation detection + moments."""
    import concourse.bacc as bacc
    import concourse.tile as tile
    from concourse import mybir

    f32 = mybir.dt.float32
    bf16 = mybir.dt.bfloat16
    mult = mybir.AluOpType.mult
    amax = mybir.AluOpType.max
    asub = mybir.AluOpType.subtract
    aadd = mybir.AluOpType.add
    Copy = mybir.ActivationFunctionType.Copy

    nc = bacc.Bacc("TRN2", target_bir_lowering=False, debug=False,
                   num_devices=N_CORES)
    f8 = mybir.dt.float8e4
    a_dram = nc.dram_tensor("a_t", [D, AR], f8, kind="ExternalInput").ap()
    b_dram = nc.dram_tensor("b_t", [D, BC], f8, kind="ExternalInput").ap()
    bsqc_dram = nc.dram_tensor("bsqc", [128, NBLK], f32,
                               kind="ExternalInput").ap()
    bexp_dram = nc.dram_tensor("bexp", [128, NBLK], f32,
                               kind="ExternalInput").ap()
    out_dram = nc.dram_tensor("out", [128, 32], f32, kind="ExternalOutput").ap()

    Exp = mybir.ActivationFunctionType.Exp
    DR = mybir.MatmulPerfMode.DoubleRow
    with tile.TileContext(nc) as tc:
        with (
            tc.tile_pool(name="big", bufs=1) as big,
            tc.tile_pool(name="junk", bufs=2) as junkp,
            tc.tile_pool(name="psum", bufs=3, space="PSUM") as psum,
        ):
            ab = big.tile([128, 2, AR], f8, tag="ab")
            bb = big.tile([128, 2, BC], f8, tag="bb")
            bsqc = big.tile([128, NBLK], f32, tag="bsqc")
            bexp = big.tile([128, NBLK], f32, tag="bexp")
            outs = big.tile([128, 32], f32, tag="outs")

            nc.gpsimd.memset(outs[:], 0.0)

            # loads: b (gates first matmul) leads the sync ring; a on scalar
            nc.sync.dma_start(bb[:], b_dram.rearrange(
                "(c p) n -> p c n", p=128, c=2))
            nc.scalar.dma_start(ab[:], a_dram.rearrange(
                "(c p) n -> p c n", p=128, c=2))
            nc.sync.dma_start(bsqc[:], bsqc_dram[:])
            nc.sync.dma_start(bexp[:], bexp_dram[:])
            a0, a1 = ab[:, 0, :], ab[:, 1, :]
            b0, b1 = bb[:, 0, :], bb[:, 1, :]

            # psum[j, i] = dot_ij; DoubleRow packs both 128-dim halves into
            # one fp8 matmul (2 weights per PE cell)
            for blk in range(NBLK):
                bs = slice(blk * 128, (blk + 1) * 128)
                ps = psum.tile([128, AR], f32, tag="ps")
                for hs in (slice(0, 512), slice(512, AR)):
                    nc.tensor.matmul(ps[:, hs], bb[:, :, bs], ab[:, :, hs],
                                     start=True, stop=True, perf_mode=DR)
                jd = junkp.tile([128, AR], f32, tag="jd")
                if blk in EXP_BLOCKS:
                    # accum[j] = sum_i exp(LAMB*(dot_ij - 0.5*sq_j - T));
                    # > 0.5 iff some element is near/inside the margin
                    nc.scalar.activation(jd[:], ps[:], Exp,
                                         bias=bexp[:, blk:blk + 1],
                                         scale=LAMB,
                                         accum_out=outs[:, blk:blk + 1])
                else:
                    # accum[j] = max( max_i(dot_ij) - 0.5*sq_j, -1.0 )
                    nc.vector.tensor_scalar(jd[:], ps[:],
                                            bsqc[:, blk:blk + 1],
                                            -1.0, asub, amax,
                                            accum_out=outs[:, blk:blk + 1])

            # ---- class-sum moments, balanced across ACT and DVE ----
            ja = junkp.tile([128, AR], bf16, tag="ja")
            nc.scalar.activation(ja[:], a0, Copy, accum_out=outs[:, 24:25])
            ja2 = junkp.tile([128, AR], bf16, tag="ja")
            nc.scalar.activation(ja2[:], a1, Copy, accum_out=outs[:, 25:26])
            jb = junkp.tile([128, BC], bf16, tag="jb")
            nc.vector.tensor_scalar(jb[:], b0, 1.0, None, mult, aadd,
                                    accum_out=outs[:, 26:27])
            jb2 = junkp.tile([128, BC], bf16, tag="jb")
            nc.vector.tensor_scalar(jb2[:], b1, 1.0, None, mult, aadd,
                                    accum_out=outs[:, 27:28])

            nc.sync.dma_start(out_dram[:], outs[:])
    nc.compile()
    return nc


def _build_full_program():
    """Exact fallback: full min/sqrt pipeline for the dissimilar term.
    Only compiled + run when the detect program finds d2_min < 1.4."""
    import concourse.bacc as bacc
    import concourse.tile as tile
    from concourse import mybir

    f32 = mybir.dt.float32
    bf16 = mybir.dt.bfloat16
    mult = mybir.AluOpType.mult
    amin = mybir.AluOpType.min
    aadd = mybir.AluOpType.add
    Sqrt = mybir.ActivationFunctionType.Sqrt

    nc = bacc.Bacc("TRN2", target_bir_lowering=False, debug=False,
                   num_devices=N_CORES)
    a_dram = nc.dram_tensor("a_t", [D + 1, AR], bf16, kind="ExternalInput").ap()
    b_dram = nc.dram_tensor("b_t", [D, BC], bf16, kind="ExternalInput").ap()
    bsqc_dram = nc.dram_tensor("bsqc", [128, NBLK], f32,
                               kind="ExternalInput").ap()
    out_dram = nc.dram_tensor("out", [128, 32], f32, kind="ExternalOutput").ap()

    with tile.TileContext(nc) as tc:
        with (
            tc.tile_pool(name="big", bufs=1) as big,
            tc.tile_pool(name="work", bufs=3) as work,
            tc.tile_pool(name="junk", bufs=2) as junkp,
            tc.tile_pool(name="psum", bufs=3, space="PSUM") as psum,
        ):
            a0 = big.tile([128, AR], bf16, tag="a0")
            a1 = big.tile([128, AR], bf16, tag="a1")
            zrow = big.tile([1, AR], bf16, tag="zrow")
            b0 = big.tile([128, BC], bf16, tag="b0")
            b1 = big.tile([128, BC], bf16, tag="b1")
            bsqc = big.tile([128, NBLK], f32, tag="bsqc")
            onesr = big.tile([1, 128], bf16, tag="onesr")
            epsb = big.tile([128, 1], f32, tag="epsb")
            cal = big.tile([1, 8], f32, tag="cal")
            outs = big.tile([128, 32], f32, tag="outs")

            nc.gpsimd.memset(outs[:], 0.0)
            nc.gpsimd.memset(onesr[:], 1.0)
            nc.gpsimd.memset(epsb[:], EPS)
            nc.gpsimd.memset(cal[:], 1.0)

            nc.sync.dma_start(a0[:], a_dram[0:128, :])
            nc.sync.dma_start(a1[:], a_dram[128:256, :])
            nc.sync.dma_start(zrow[:], a_dram[256:257, :])
            nc.sync.dma_start(b0[:], b_dram[0:128, :])
            nc.sync.dma_start(b1[:], b_dram[128:256, :])
            nc.sync.dma_start(bsqc[:], bsqc_dram[:])

            for blk in range(NBLK):
                bs = slice(blk * 128, (blk + 1) * 128)
                ps = psum.tile([128, AR], f32, tag="ps")
                for hs in (slice(0, 512), slice(512, AR)):
                    nc.tensor.matmul(ps[:, hs], b0[:, bs], a0[:, hs],
                                     start=True, stop=False)
                    nc.tensor.matmul(ps[:, hs], b1[:, bs], a1[:, hs],
                                     start=False, stop=False)
                    nc.tensor.matmul(ps[:, hs], onesr[:], zrow[:, hs],
                                     start=False, stop=True)
                # t = min(d2, 1) = min(-2*(psum - 0.5*sqb_j), 1)
                #   = -2 * max(psum - 0.5*sqb_j, -0.5)
                u = work.tile([128, AR], f32, tag="u")
                nc.vector.tensor_scalar(u[:], ps[:], bsqc[:, blk:blk + 1],
                                        -0.5, mybir.AluOpType.subtract,
                                        mybir.AluOpType.max)
                t = work.tile([128, AR], bf16, tag="t")
                nc.vector.tensor_scalar(t[:], u[:], -2.0, None, mult, aadd,
                                        accum_out=outs[:, 8 + blk:9 + blk])
                # s = sqrt(t + eps); accum = row sums
                sj = work.tile([128, AR], bf16, tag="sj")
                nc.scalar.activation(sj[:], t[:], Sqrt, bias=epsb[:],
                                     scale=1.0,
                                     accum_out=outs[:, blk:blk + 1])
            # calibration: s1_hat = ACT_sqrt(1 + eps) summed over 8 ones
            jcal = junkp.tile([1, 8], f32, tag="jcal")
            nc.scalar.activation(jcal[:], cal[:], Sqrt, bias=epsb[0:1, :],
                                 scale=1.0, accum_out=outs[0:1, 29:30])

            nc.sync.dma_start(out_dram[:], outs[:])
    nc.compile()
    return nc


def _get_program(kind):
    if kind not in _PROGRAMS:
        _PROGRAMS[kind] = (_build_detect_program() if kind == "detect"
                           else _build_full_program())
    return _PROGRAMS[kind]


def build_in_maps(emb, lab):
    """Host-side sharding prep. Returns (in_maps, meta) or None if the
    label split exceeds the compiled caps."""
    import ml_dtypes
    bf16 = ml_dtypes.bfloat16

    idx0 = np.nonzero(lab == 0)[0]
    idx1 = np.nonzero(lab == 1)[0]
    if len(idx0) <= len(idx1):
        idxs, idxl = idx0, idx1
    else:
        idxs, idxl = idx1, idx0
    ns, nl = len(idxs), len(idxl)
    if ns > R_CAP or nl > C_CAP:
        return None
    Es = emb[idxs]                      # (ns, 256)  -> rectangle rows (free)
    El = emb[idxl]                      # (nl, 256)  -> rectangle cols (parts)
    sqs = np.einsum('ij,ij->i', Es.astype(np.float64), Es.astype(np.float64))
    sql = np.einsum('ij,ij->i', El.astype(np.float64), El.astype(np.float64))

    # a side: embeddings + z row  (z = -0.5*(sq + pad_bias), full prog only)
    A = np.zeros((D + 1, R_CAP), np.float32)
    A[:D, :ns] = Es.T
    A[D, :ns] = (-0.5 * sqs).astype(np.float32)
    A[D, ns:] = -0.5 * BIG

    # b side: embeddings only; its sq goes in per-partition columns
    Bt = np.zeros((D, C_CAP), np.float32)
    Bt[:, :nl] = El.T
    bsq_flat = np.full((C_CAP,), 0.5 * BIG, np.float32)
    bsq_flat[:nl] = (0.5 * sql).astype(np.float32)

    f8 = ml_dtypes.float8_e4m3
    A_bf = A.astype(bf16)
    Bt_bf = Bt.astype(bf16)
    A_f8 = A[:D].astype(f8)
    Bt_f8 = Bt.astype(f8)

    sqmin_a = float(sqs.min()) if ns else float("inf")
    T = DETECT_ACCUM_THRESH + 0.5 * sqmin_a - 3.0
    bexp_flat = -LAMB * (bsq_flat.astype(np.float64) + T)

    in_maps = []
    for ri in range(RSH):
        for ci in range(CSH):
            bslice = bsq_flat[ci * BC:(ci + 1) * BC]
            eslice = bexp_flat[ci * BC:(ci + 1) * BC]
            in_maps.append({
                "a_t": np.ascontiguousarray(A_f8[:, ri * AR:(ri + 1) * AR]),
                "b_t8": np.ascontiguousarray(
                    Bt_f8[:, ci * BC:(ci + 1) * BC]),
                "a_tz": np.ascontiguousarray(A_bf[:, ri * AR:(ri + 1) * AR]),
                "b_t": np.ascontiguousarray(Bt_bf[:, ci * BC:(ci + 1) * BC]),
                "bsqc": np.ascontiguousarray(
                    bslice.reshape(NBLK, 128).T.astype(np.float32)),
                "bexp": np.ascontiguousarray(
                    eslice.reshape(NBLK, 128).T.astype(np.float32)),
            })
    meta = (ns, nl, float(sqs.sum()), float(sql.sum()), sqmin_a)
    return in_maps, meta


def combine_term1(outs_list, ns, nl, sum_sq_small, sum_sq_large):
    """Similar-pair closed form: device class sums + host sq sums (float64)."""
    o = [np.asarray(x, np.float64) for x in outs_list]
    S_small = np.zeros(D)
    for ri in range(RSH):
        ok = o[ri * CSH + 0]
        S_small[0:128] += ok[:, 24]
        S_small[128:256] += ok[:, 25]
    S_large = np.zeros(D)
    for ci in range(CSH):
        ok = o[ci]
        S_large[0:128] += ok[:, 26]
        S_large[128:256] += ok[:, 27]
    term1_d2 = (ns * sum_sq_small - S_small @ S_small
                + nl * sum_sq_large - S_large @ S_large)
    n_same = ns * (ns - 1) / 2.0 + nl * (nl - 1) / 2.0
    return 0.5 * (term1_d2 + EPS * n_same)


def combine_term2_full(outs_list):
    """Dissimilar term from the full program's accumulators (float64)."""
    o = [np.asarray(x, np.float64) for x in outs_list]
    n_elem = float(R_CAP) * float(C_CAP)
    Ts = sum(ok[:, 0:NBLK].sum() for ok in o)          # sum of sqrt(t+eps)
    Tt = sum(ok[:, 8:8 + NBLK].sum() for ok in o)      # sum of t
    s1_hat = o[0][0, 29] / 8.0
    return 0.5 * ((Tt - n_elem) + 2.0 * (n_elem * s1_hat - Ts))


def _numpy_fallback(emb, lab):
    e = emb.astype(np.float64)
    sq = (e * e).sum(1)
    gram = e @ e.T
    d2 = np.maximum(sq[:, None] + sq[None, :] - 2.0 * gram, 0.0)
    dist = np.sqrt(d2 + EPS)
    same = (lab[:, None] == lab[None, :]).astype(np.float64)
    loss = same * 0.5 * dist ** 2 \
        + (1.0 - same) * 0.5 * np.maximum(MARGIN - dist, 0.0) ** 2
    mask = np.triu(np.ones_like(loss), k=1)
    return (loss * mask).sum() / (mask.sum() + EPS)


def run_device(in_maps, kind="detect", trace=False, **kw):
    from concourse.bass_utils import run_bass_kernel_spmd
    nc = _get_program(kind)
    if kind == "detect":
        maps = [{"a_t": m["a_t"], "b_t": m["b_t8"], "bsqc": m["bsqc"],
                 "bexp": m["bexp"]} for m in in_maps]
    else:
        maps = [{"a_t": m["a_tz"], "b_t": m["b_t"], "bsqc": m["bsqc"]}
                for m in in_maps]
    return run_bass_kernel_spmd(nc, maps, list(range(N_CORES)),
                                trace=trace, **kw)


def kernel(embeddings, labels):
    emb = np.ascontiguousarray(np.asarray(embeddings), dtype=np.float32)
    lab = np.asarray(labels).astype(np.int64).ravel()
    ok_shapes = (emb.shape == (B_FULL, D) and lab.shape == (B_FULL,)
                 and np.all((lab == 0) | (lab == 1)))
    prep = build_in_maps(emb, lab) if ok_shapes else None
    if prep is None:
        return np.float32(_numpy_fallback(emb, lab))
    in_maps, (ns, nl, ssq_s, ssq_l, sqmin_a) = prep

    res = run_device(in_maps, kind="detect")
    outs_list = [res.results[k]["out"] for k in range(N_CORES)]
    term1 = combine_term1(outs_list, ns, nl, ssq_s, ssq_l)

    # MAX blocks: accum[j] = max_i(dot_ij) - 0.5*sq_j; a pair with d2 < 1.4
    # forces accum[j] > T = -0.7 + 0.5*min_i(sq_i) (3.0 slack for fp8).
    # EXP blocks: accum[j] = sum_i exp(LAMB*(dot - 0.5*sq_j - T)) > 0.5.
    T = DETECT_ACCUM_THRESH + 0.5 * sqmin_a - 3.0
    mx = max(float(ok[:, list(MAX_BLOCKS)].max()) for ok in outs_list)
    ex = max(float(np.nan_to_num(ok[:, list(EXP_BLOCKS)], nan=1e30).max())
             for ok in outs_list)
    if ns > 0 and (mx > T or ex > 0.5):
        # some mixed pair may be near/inside the margin: exact slow path
        res2 = run_device(in_maps, kind="full")
        term2 = combine_term2_full(
            [res2.results[k]["out"] for k in range(N_CORES)])
    else:
        term2 = 0.0

    den = B_FULL * (B_FULL - 1) / 2.0 + EPS
    return np.float32((term1 + term2) / den)



# revision 4
# speedup vs baseline: 1.0729x; 1.0729x over previous
"""Trainium2 Bass kernel for nn_ContrastiveLoss (B=4096, D=256, margin=1.0).

Math (exact restructuring of the reference):
  loss = [ sum_{i<j, same} 0.5*(d2_ij + 1e-8)
         + sum_{i<j, diff} 0.5*relu(1 - d_ij)^2 ] / (B(B-1)/2 + 1e-8)

  Similar-pair term has a closed form per class c:
     sum_{i<j in c} d2 = n_c * sum_sq_c - ||sum_e_c||^2
  needing only class sums (device) and squared-norm sums (host fp64).

  The dissimilar term needs elementwise work only on the mixed-label
  rectangle, and relu(1-d)^2 is EXACTLY zero unless some mixed pair has
  d2 < 1.  The device program PROVES no pair violates the margin: an fp8
  DoubleRow GEMM leaves psum[j,i] = dot_ij and
    - DVE max-reduce emits max_i dot_ij per j  (host applies threshold)
    - ACT relu-sum emits sum_i relu(dot_ij - 0.5*sq_j - T) per j
  If no chunk shows a value above threshold, the dissimilar term is
  exactly 0.  Otherwise a host fp64 fallback recomputes the loss exactly.

Sharding: the LARGE class is the GEMM free axis (2 row-shards of 1040,
split 512/528), the SMALL class is the psum partition axis (4 col-shards
of 512 = 4 blocks of 128).  8 cores = 2x4 grid.  Each core also sums a
disjoint quarter of the class-sum moments from a dedicated zero-masked
blob.  Inputs are packed on host into SBUF-layout blobs so each DMA is
128 contiguous ~1KB rows.  A junk-matmul warmup during the DMA wait
releases the PE HAM clock gate (idle PE runs 1.2 GHz, busy 2.4 GHz).
"""

import sys
import os

for _p in ("/opt/trn_rl_repo", "/root/.axon_site/_ro/trn_rl_repo"):
    if os.path.isdir(_p) and _p not in sys.path:
        sys.path.insert(0, _p)

import numpy as np

B_FULL, D = 4096, 256
MARGIN = 1.0
EPS = 1e-8
BIG = 1.0e4
RSH, CSH = 2, 4                # core grid: a(row)-shards x b(col)-shards
A_CAP = 2080                   # padded large-class size (free axis)
AR = A_CAP // RSH              # 1040 free cols per core
H0, H1 = 512, 528              # free-axis halves (bank-aligned first chunk)
B_CAP = 2048                   # padded small-class size (partition axis)
BC = B_CAP // CSH              # 512 psum columns per core
NBLK = BC // 128               # 4 psum blocks per core
N_CORES = RSH * CSH

# per-core moment blob layout: [a_c0 | a_c1 | b_c0 | b_c1] (fp8 bytes)
MQ = 272                       # max a-shard quarter width
MB = BC // RSH                 # b-shard half width (256)
QBOUND = (0, 260, 512, 784, 1040)   # a-shard quarter boundaries

# detection: trigger the exact fallback if min mixed d2 could be < 1.4
DETECT_THRESH = -0.7
FP8_SLACK = 3.0
DVE_ORDER = ((0, 0), (2, 0), (0, 1), (2, 1), (3, 1))  # (blk, h) max-reduce
ACT_CHUNKS = ((1, 0), (3, 0), (1, 1))                 # (blk, h) relu-sum
N_WARMUP_MM = 22

_PROGRAMS = {}


def _build_detect_program():
    import concourse.bacc as bacc
    import concourse.tile as tile
    from concourse import mybir

    f32 = mybir.dt.float32
    bf16 = mybir.dt.bfloat16
    f8 = mybir.dt.float8e4
    mult = mybir.AluOpType.mult
    aadd = mybir.AluOpType.add
    amax = mybir.AluOpType.max
    AxX = mybir.AxisListType.X
    Relu = mybir.ActivationFunctionType.Relu
    DR = mybir.MatmulPerfMode.DoubleRow

    nc = bacc.Bacc("TRN2", target_bir_lowering=False, debug=False,
                   num_devices=N_CORES)
    a_dram = nc.dram_tensor("a_t", [128, 2 * AR], f8, kind="ExternalInput").ap()
    b_dram = nc.dram_tensor("b_t", [128, 2 * BC], f8, kind="ExternalInput").ap()
    m_dram = nc.dram_tensor("mom", [128, 2 * (MQ + MB)], f8,
                            kind="ExternalInput").ap()
    c_dram = nc.dram_tensor("cst", [128, len(ACT_CHUNKS)], f32,
                            kind="ExternalInput").ap()
    out_dram = nc.dram_tensor("out", [128, 12], f32, kind="ExternalOutput").ap()

    with tile.TileContext(nc) as tc:
        with (
            tc.tile_pool(name="big", bufs=1) as big,
            tc.tile_pool(name="junk", bufs=2) as junkp,
            tc.tile_pool(name="ps0", bufs=2, space="PSUM") as psum0,
            tc.tile_pool(name="ps1", bufs=3, space="PSUM") as psum1,
        ):
            ab0 = big.tile([128, 2, H0], f8, tag="ab0")
            ab1 = big.tile([128, 2, H1], f8, tag="ab1")
            bb = big.tile([128, 2, BC], f8, tag="bb")
            mom = big.tile([128, 2 * (MQ + MB)], f8, tag="mom")
            cst = big.tile([128, len(ACT_CHUNKS)], f32, tag="cst")
            outs = big.tile([128, 12], f32, tag="outs")
            junk_w = big.tile([128, 2, 128], f8, tag="junk_w")

            # warmup weights memset leads the gpsimd queue
            nc.gpsimd.memset(junk_w[:], 0.0)

            # input DMAs: b + consts + moments on the sync ring, a on scalar
            nc.sync.dma_start(bb[:], b_dram[:])
            nc.sync.dma_start(cst[:], c_dram[:])
            nc.sync.dma_start(mom[:], m_dram[:])
            nc.scalar.dma_start(ab0[:], a_dram[:, 0:2 * H0])
            nc.scalar.dma_start(ab1[:], a_dram[:, 2 * H0:2 * AR])

            # PE warmup: junk DR matmuls keep the HAM clock gate open while
            # the input DMAs land
            wps = psum0.tile([128, H0], f32, tag="ps0")
            for _ in range(N_WARMUP_MM):
                nc.tensor.matmul(wps[:, 0:128], junk_w[:], junk_w[:],
                                 start=True, stop=True, perf_mode=DR)

            # GEMM chunks: h0 pass then h1 pass (weights reloaded per pass)
            ps_of = {}
            for blk in range(NBLK):
                bs = slice(blk * 128, (blk + 1) * 128)
                ps = psum0.tile([128, H0], f32, tag="ps0")
                nc.tensor.matmul(ps[:], bb[:, :, bs], ab0[:],
                                 start=True, stop=True, perf_mode=DR)
                ps_of[(blk, 0)] = ps
            for blk in range(NBLK):
                bs = slice(blk * 128, (blk + 1) * 128)
                ps = psum1.tile([128, 1024], f32, tag="ps1")
                nc.tensor.matmul(ps[:, 0:H0], bb[:, :, bs], ab1[:, :, 0:H0],
                                 start=True, stop=True, perf_mode=DR)
                nc.tensor.matmul(ps[:, H0:H1], bb[:, :, bs], ab1[:, :, H0:H1],
                                 start=True, stop=True, perf_mode=DR)
                ps_of[(blk, 1)] = ps

            # detection: DVE raw max-reduce (host subtracts 0.5*sq_j and
            # compares to T); ACT relu-sum with per-partition bias
            for k, (blk, h) in enumerate(DVE_ORDER):
                ps = ps_of[(blk, h)]
                w = H0 if h == 0 else H1
                nc.vector.tensor_reduce(outs[:, k:k + 1], ps[:, 0:w],
                                        AxX, amax)
            for k, (blk, h) in enumerate(ACT_CHUNKS):
                ps = ps_of[(blk, h)]
                w = H0 if h == 0 else H1
                ja = junkp.tile([128, H1], bf16, tag="ja")
                nc.scalar.activation(ja[:, 0:w], ps[:, 0:w], Relu,
                                     bias=cst[:, k:k + 1], scale=1.0,
                                     accum_out=outs[:, 5 + k:6 + k])

            # class-sum moments from the per-core zero-masked blob:
            # a-quarter sums on DVE reduce-add, b-half sums on ACT accum
            Copy = mybir.ActivationFunctionType.Copy
            for c in range(2):
                nc.vector.tensor_reduce(outs[:, 8 + c:9 + c],
                                        mom[:, c * MQ:(c + 1) * MQ],
                                        AxX, aadd)
            for c in range(2):
                jm = junkp.tile([128, MB], bf16, tag="jm")
                lo = 2 * MQ + c * MB
                nc.scalar.activation(jm[:], mom[:, lo:lo + MB], Copy,
                                     accum_out=outs[:, 10 + c:11 + c])

            nc.sync.dma_start(out_dram[:], outs[:])
    nc.compile()
    return nc


def _get_program(kind):
    if kind not in _PROGRAMS:
        _PROGRAMS[kind] = _build_detect_program()
    return _PROGRAMS[kind]


def build_in_maps(emb, lab):
    """Host-side prep. Returns (in_maps, meta) or None if caps exceeded."""
    import ml_dtypes
    f8 = ml_dtypes.float8_e4m3

    idx0 = np.nonzero(lab == 0)[0]
    idx1 = np.nonzero(lab == 1)[0]
    if len(idx0) <= len(idx1):
        idxs, idxl = idx0, idx1
    else:
        idxs, idxl = idx1, idx0
    ns, nl = len(idxs), len(idxl)
    if ns > B_CAP or nl > A_CAP:
        return None
    Es = emb[idxs]                      # (ns, 256) small -> psum partitions
    El = emb[idxl]                      # (nl, 256) large -> free axis
    sqs = np.einsum('ij,ij->i', Es.astype(np.float64), Es.astype(np.float64))
    sql = np.einsum('ij,ij->i', El.astype(np.float64), El.astype(np.float64))

    A = np.zeros((D, A_CAP), np.float32)
    A[:, :nl] = El.T
    Bt = np.zeros((D, B_CAP), np.float32)
    Bt[:, :ns] = Es.T
    A_f8 = A.astype(f8)
    B_f8 = Bt.astype(f8)

    sq_b = np.full((B_CAP,), BIG, np.float64)
    sq_b[:ns] = sqs

    sqmin_a = float(sql.min()) if nl else float("inf")
    T = DETECT_THRESH + 0.5 * sqmin_a - FP8_SLACK

    in_maps = []
    for ri in range(RSH):
        base = ri * AR
        # a blob row layout: [h0: c0 512 | c1 512][h1: c0 528 | c1 528]
        a_blob = np.zeros((128, 2 * AR), f8)
        a_blob[:, 0:H0] = A_f8[0:128, base:base + H0]
        a_blob[:, H0:2 * H0] = A_f8[128:256, base:base + H0]
        a_blob[:, 2 * H0:2 * H0 + H1] = A_f8[0:128, base + H0:base + AR]
        a_blob[:, 2 * H0 + H1:2 * AR] = A_f8[128:256, base + H0:base + AR]
        for ci in range(CSH):
            cb = ci * BC
            b_blob = np.zeros((128, 2 * BC), f8)
            b_blob[:, 0:BC] = B_f8[0:128, cb:cb + BC]
            b_blob[:, BC:2 * BC] = B_f8[128:256, cb:cb + BC]
            cst = np.zeros((128, len(ACT_CHUNKS)), np.float32)
            for k, (blk, _h) in enumerate(ACT_CHUNKS):
                cst[:, k] = (-(0.5 * sq_b[cb + blk * 128:cb + (blk + 1) * 128]
                               + T)).astype(np.float32)
            # moment blob: this core's a-shard quarter + b-shard half,
            # zero-padded to the fixed [a_c0|a_c1|b_c0|b_c1] layout
            lo, hi = QBOUND[ci], QBOUND[ci + 1]
            m_blob = np.zeros((128, 2 * (MQ + MB)), f8)
            m_blob[:, 0:hi - lo] = A_f8[0:128, base + lo:base + hi]
            m_blob[:, MQ:MQ + (hi - lo)] = A_f8[128:256, base + lo:base + hi]
            hb = ri * MB
            m_blob[:, 2 * MQ:2 * MQ + MB] = B_f8[0:128, cb + hb:cb + hb + MB]
            m_blob[:, 2 * MQ + MB:] = B_f8[128:256, cb + hb:cb + hb + MB]
            in_maps.append({
                "a_t": np.ascontiguousarray(a_blob),
                "b_t": np.ascontiguousarray(b_blob),
                "mom": np.ascontiguousarray(m_blob),
                "cst": np.ascontiguousarray(cst),
            })
    meta = (ns, nl, float(sqs.sum()), float(sql.sum()), sqmin_a, sq_b)
    return in_maps, meta


def combine_term1(outs_list, ns, nl, ssq_s, ssq_l):
    """Similar-pair closed form (float64): device class sums + host sq."""
    o = [np.asarray(x, np.float64) for x in outs_list]
    S_l = np.zeros(D)
    S_s = np.zeros(D)
    for k in range(N_CORES):
        for c in range(2):
            S_l[c * 128:(c + 1) * 128] += o[k][:, 8 + c]
            S_s[c * 128:(c + 1) * 128] += o[k][:, 10 + c]
    term1_d2 = (ns * ssq_s - S_s @ S_s + nl * ssq_l - S_l @ S_l)
    n_same = ns * (ns - 1) / 2.0 + nl * (nl - 1) / 2.0
    return 0.5 * (term1_d2 + EPS * n_same)


def _numpy_fallback(emb, lab):
    e = emb.astype(np.float64)
    sq = (e * e).sum(1)
    gram = e @ e.T
    d2 = np.maximum(sq[:, None] + sq[None, :] - 2.0 * gram, 0.0)
    dist = np.sqrt(d2 + EPS)
    same = (lab[:, None] == lab[None, :]).astype(np.float64)
    loss = same * 0.5 * dist ** 2 \
        + (1.0 - same) * 0.5 * np.maximum(MARGIN - dist, 0.0) ** 2
    mask = np.triu(np.ones_like(loss), k=1)
    return (loss * mask).sum() / (mask.sum() + EPS)


def run_device(in_maps, kind="detect", trace=False, **kw):
    from concourse.bass_utils import run_bass_kernel_spmd
    nc = _get_program(kind)
    maps = [{"a_t": m["a_t"], "b_t": m["b_t"], "mom": m["mom"],
             "cst": m["cst"]} for m in in_maps]
    return run_bass_kernel_spmd(nc, maps, list(range(N_CORES)),
                                trace=trace, **kw)


def kernel(embeddings, labels):
    emb = np.ascontiguousarray(np.asarray(embeddings), dtype=np.float32)
    lab = np.asarray(labels).astype(np.int64).ravel()
    ok_shapes = (emb.shape == (B_FULL, D) and lab.shape == (B_FULL,)
                 and np.all((lab == 0) | (lab == 1)))
    prep = build_in_maps(emb, lab) if ok_shapes else None
    if prep is None:
        return np.float32(_numpy_fallback(emb, lab))
    in_maps, (ns, nl, ssq_s, ssq_l, sqmin_a, sq_b) = prep

    res = run_device(in_maps, kind="detect")
    outs_list = [np.asarray(res.results[k]["out"], np.float64)
                 for k in range(N_CORES)]
    term1 = combine_term1(outs_list, ns, nl, ssq_s, ssq_l)

    # margin-violation certificate
    T = DETECT_THRESH + 0.5 * sqmin_a - FP8_SLACK
    trigger = False
    if ns > 0 and nl > 0:
        for core in range(N_CORES):
            ci = core % CSH
            ok = outs_list[core]
            for k, (blk, _h) in enumerate(DVE_ORDER):
                sqj = sq_b[ci * BC + blk * 128:ci * BC + (blk + 1) * 128]
                if np.any(np.nan_to_num(ok[:, k], nan=1e30) - 0.5 * sqj > T):
                    trigger = True
            if np.any(np.nan_to_num(ok[:, 5:8], nan=1e30) > 0.1):
                trigger = True
    if trigger:
        return np.float32(_numpy_fallback(emb, lab))

    den = B_FULL * (B_FULL - 1) / 2.0 + EPS
    return np.float32(term1 / den)
